# revision 1
# baseline (speedup 1.0000x reference)
"""CausalTemporalAttnBlock Trainium2 kernel.

Problem: out = x + Wp @ attn(norm(x)) + bp, where norm is GroupNorm(1 group)
over (c,t,h,w) per batch, attention is causal over t, independent per (b,h,w).
Shapes: x (2, 512, 64, 32, 32) fp32; four (512,512) weights + biases.

Strategy (8 NeuronCores, zero communication except a 4-float AllReduce for
the GroupNorm stats):
  - core i handles batch i//4, h-rows [8*(i%4), 8*(i%4)+8), all w: 256 (h,w)
    locations per core.
  - Host folds gamma/beta/mean/rstd into the projection weights:
        q = r*(Aq @ x) + (cq - mu*r*uq),   Aq = wq*diag(gamma) (pre-scaled by
    1/sqrt(c) for q), uq = wq@gamma, cq = bq + wq@beta. mu, r=rstd computed
    on device (AllReduce of per-batch sum/sumsq across the 4 cores of each
    batch); the affine is applied at PSUM-eviction time.
  - Host re-lays the shard as [8 h-rows][512 c][64 t * 32 w] so every DMA is
    >=8KB contiguous.
  - Per h-row block (32 locs), per group of 8 locs: Q/K projections
    (c-on-partitions), V produced transposed (VT, t-on-partitions) directly
    by making x the stationary operand, scores computed transposed
    S^T = K^T Q (s-on-partitions) so softmax normalization is a ones-matmul
    and AV needs no transposes at all. No max-subtraction (scores are O(1);
    exp is safe in fp32). Causal mask applied as a 0/1 multiply after exp.
  - All big matmuls use float32r (full PE rate at N>=512, ~fp32 accuracy).
"""

import numpy as np

import concourse.bass as bass
import concourse.tile as tile
from concourse import bacc, mybir
from concourse.bass_utils import run_bass_kernel_spmd

P = 128
B, C, T, H, W = 2, 512, 64, 32, 32
NCORES = 8
HSH = H // 4          # 8 h-rows per core
CCH = C // P          # 4 c chunks
GRP = 8               # locations per attention group
NGRP = W // GRP       # 4 groups per block
EPS = 1e-6

f32 = mybir.dt.float32
f32r = mybir.dt.float32r
AX = mybir.AxisListType.X
ALU = mybir.AluOpType
AF = mybir.ActivationFunctionType


def build_nc(num_cores=NCORES, nblk=HSH, norm_n=None, replica_groups=None,
             reps=1, use_collective=True):
    if norm_n is None:
        norm_n = C * T * H * W
    if replica_groups is None:
        replica_groups = [[0, 1, 2, 3], [4, 5, 6, 7]]
    nc = bacc.Bacc("TRN2", target_bir_lowering=False, debug=False,
                   num_devices=num_cores)

    xs = nc.declare_dram_parameter("xs", [nblk, C, T * W], f32r, isOutput=False)
    wts = {}
    for nm in ("q", "k", "v", "p"):
        wts[nm] = nc.declare_dram_parameter(f"w{nm}t", [C, C], f32r,
                                            isOutput=False)
    ucol = nc.declare_dram_parameter("ucol", [P, 2 * CCH], f32, isOutput=False)
    ccol = nc.declare_dram_parameter("ccol", [P, 2 * CCH], f32, isOutput=False)
    uvrow = nc.declare_dram_parameter("uvrow", [1, C], f32, isOutput=False)
    cvrow = nc.declare_dram_parameter("cvrow", [1, C], f32, isOutput=False)
    bprow = nc.declare_dram_parameter("bprow", [1, C], f32r, isOutput=False)
    maskp = nc.declare_dram_parameter("maskt", [T, GRP * T], f32, isOutput=False)
    ones_col_f = nc.declare_dram_parameter("ones_col_f", [P, 1], f32, isOutput=False)
    ones_col_r = nc.declare_dram_parameter("ones_col_r", [P, 1], f32r, isOutput=False)
    ones_row_r = nc.declare_dram_parameter("ones_row_r", [1, C], f32r, isOutput=False)
    outp = nc.declare_dram_parameter("out", [nblk, C, T * W], f32,
                                     isOutput=True)
    cc_in = nc.dram_tensor("cc_in", [1, 2], f32)
    cc_out = nc.dram_tensor("cc_out", [1, 2], f32)

    with tile.TileContext(nc) as tc:
        with (
            tc.tile_pool(name="const", bufs=1) as const,
            tc.tile_pool(name="scal", bufs=1) as sc,
            tc.tile_pool(name="statp", bufs=2) as statp,
            tc.tile_pool(name="xpool", bufs=2) as xpool,
            tc.tile_pool(name="gpool", bufs=8) as gpool,
            tc.tile_pool(name="spool", bufs=2) as spool,
            tc.tile_pool(name="pp", bufs=3, space="PSUM") as pp,
            tc.tile_pool(name="pss", bufs=2, space="PSUM") as pss,
            tc.tile_pool(name="psm", bufs=1, space="PSUM") as psm,
        ):
            # ---------- constants ----------
            w_sb = {}
            for nm in ("q", "k", "v", "p"):
                for ci in range(CCH):
                    t = const.tile([P, C], f32r, tag=f"w{nm}{ci}")
                    nc.sync.dma_start(t[:], wts[nm][ci * P:(ci + 1) * P, :])
                    w_sb[nm, ci] = t
            ucol_sb = const.tile([P, 2 * CCH], f32, tag="ucol")
            nc.sync.dma_start(ucol_sb[:], ucol[:])
            ccol_sb = const.tile([P, 2 * CCH], f32, tag="ccol")
            nc.sync.dma_start(ccol_sb[:], ccol[:])
            uvrow_sb = const.tile([1, C], f32, tag="uvrow")
            nc.sync.dma_start(uvrow_sb[:], uvrow[:])
            cvrow_sb = const.tile([1, C], f32, tag="cvrow")
            nc.sync.dma_start(cvrow_sb[:], cvrow[:])
            bprow_sb = const.tile([1, C], f32r, tag="bprow")
            nc.sync.dma_start(bprow_sb[:], bprow[:])
            mask_sb = const.tile([T, GRP * T], f32, tag="maskt")
            nc.sync.dma_start(mask_sb[:], maskp[:])
            ocf_sb = const.tile([P, 1], f32, tag="ocf")
            nc.sync.dma_start(ocf_sb[:], ones_col_f[:])
            ocr_sb = const.tile([P, 1], f32r, tag="ocr")
            nc.sync.dma_start(ocr_sb[:], ones_col_r[:])
            orr_sb = const.tile([1, C], f32r, tag="orr")
            nc.sync.dma_start(orr_sb[:], ones_row_r[:])

            # repeat body for timing variants (reps>1)
            for _rep in range(reps):
                # ---------- stats ----------
                ssum = sc.tile([P, nblk * CCH], f32, tag="ssum")
                ssq = sc.tile([P, nblk * CCH], f32, tag="ssq")
                for blk in range(nblk):
                    for ci in range(CCH):
                        xt = statp.tile([P, T * W], f32, tag="xstat")
                        nc.sync.dma_start(
                            xt[:], xs[blk, ci * P:(ci + 1) * P, :].bitcast(f32))
                        i = blk * CCH + ci
                        nc.vector.reduce_sum(out=ssum[:, i:i + 1], in_=xt[:],
                                             axis=AX)
                        # tensor_tensor_reduce faults on this HW/runtime; square
                        # in place on ACT, then a plain DVE reduction
                        nc.scalar.activation(xt[:], xt[:], AF.Square)
                        nc.vector.reduce_sum(out=ssq[:, i:i + 1], in_=xt[:],
                                             axis=AX)
                st2 = sc.tile([P, 2], f32, tag="st2")
                nc.vector.reduce_sum(out=st2[:, 0:1], in_=ssum[:], axis=AX)
                nc.vector.reduce_sum(out=st2[:, 1:2], in_=ssq[:], axis=AX)
                ps_small = psm.tile([P, 512], f32, tag="psmall")
                nc.tensor.matmul(ps_small[0:1, 0:2], ocf_sb[:], st2[:],
                                 start=True, stop=True)
                st_sb = sc.tile([1, 2], f32, tag="st_sb")
                nc.vector.tensor_copy(st_sb[:], ps_small[0:1, 0:2])
                nc.gpsimd.dma_start(cc_in[:], st_sb[:])
                if use_collective:
                    nc.gpsimd.collective_compute(
                        "AllReduce", ALU.add, replica_groups=replica_groups,
                        ins=[cc_in[:]], outs=[cc_out[:]])
                else:
                    nc.gpsimd.dma_start(cc_out[:], cc_in[:])
                stg = sc.tile([1, 2], f32, tag="stg")
                nc.gpsimd.dma_start(stg[:], cc_out[:])

                mean = sc.tile([1, 1], f32, tag="mean")
                nc.scalar.activation(mean[:], stg[:, 0:1], AF.Copy,
                                     bias=0.0, scale=1.0 / norm_n)
                ex2 = sc.tile([1, 1], f32, tag="ex2")
                nc.scalar.activation(ex2[:], stg[:, 1:2], AF.Copy,
                                     bias=0.0, scale=1.0 / norm_n)
                msq = sc.tile([1, 1], f32, tag="msq")
                nc.scalar.activation(msq[:], mean[:], AF.Square)
                varp = sc.tile([1, 1], f32, tag="varp")
                nc.vector.tensor_scalar(varp[:], ex2[:], msq[:], EPS,
                                        ALU.subtract, ALU.add)
                sqv = sc.tile([1, 1], f32, tag="sqv")      # = 1/rstd
                nc.scalar.activation(sqv[:], varp[:], AF.Sqrt)
                rst = sc.tile([1, 1], f32, tag="rst")      # = rstd
                nc.vector.reciprocal(rst[:], sqv[:])
                rmu = sc.tile([1, 1], f32, tag="rmu")      # = rstd*mean
                nc.vector.tensor_scalar(rmu[:], mean[:], rst[:], None, ALU.mult)
                vals = sc.tile([1, 2], f32r, tag="vals")
                nc.vector.tensor_copy(vals[:, 0:1], rst[:])
                nc.vector.tensor_copy(vals[:, 1:2], rmu[:])
                # broadcast (rstd, rstd*mean) across 128 partitions via K=1 matmul
                nc.tensor.matmul(ps_small[:, 0:2], orr_sb[0:1, 0:P], vals[:],
                                 start=True, stop=True)
                rb = sc.tile([P, 2], f32, tag="rb")
                nc.vector.tensor_copy(rb[:], ps_small[:, 0:2])
                # per-(proj,chunk) eviction biases for q,k: D = ccol - rmu*ucol
                dcol = sc.tile([P, 2 * CCH], f32, tag="dcol")
                nc.vector.tensor_scalar(dcol[:], ucol_sb[:], rb[:, 1:2], None,
                                        ALU.mult)
                nc.vector.tensor_sub(dcol[:], ccol_sb[:], dcol[:])
                # VT rank-1 row: dvr = (cvrow - rmu*uvrow) / rstd
                tv0 = sc.tile([1, C], f32, tag="tv0")
                nc.vector.tensor_scalar(tv0[:], uvrow_sb[:], rmu[:], None,
                                        ALU.mult)
                nc.vector.tensor_sub(tv0[:], cvrow_sb[:], tv0[:])
                dvr = sc.tile([1, C], f32r, tag="dvr")
                nc.vector.tensor_scalar(dvr[:], tv0[:], sqv[:], None, ALU.mult)

                # ---------- main blocks ----------
                for blk in range(nblk):
                    xb = []
                    for ci in range(CCH):
                        t = xpool.tile([P, T * W], f32r, tag=f"xb{ci}")
                        nc.sync.dma_start(t[:], xs[blk, ci * P:(ci + 1) * P, :])
                        xb.append(t)

                    def xgrp(ci, w0, n=GRP):
                        # [128, w(n) x t(64)] view of group cols, w-major
                        return xb[ci][:].rearrange(
                            "p (t w) -> p w t", w=W)[:, w0:w0 + n, :]

                    def xloc(ci, w):
                        # [128, t(64)] stationary view for VT production
                        return xb[ci][:].rearrange(
                            "p (t w) -> p w t", w=W)[:, w, :]

                    for g in range(NGRP):
                        w0 = g * GRP
                        # ---- Q, K projections: psum[co, (t,w)] over ci ----
                        qk = {}
                        for pi, nm in enumerate(("q", "k")):
                            for co in range(CCH):
                                ps = pp.tile([P, 512], f32, tag="pp")
                                for ci in range(CCH):
                                    nc.tensor.matmul(
                                        ps[:], w_sb[nm, ci][:, co * P:(co + 1) * P],
                                        xgrp(ci, w0), start=(ci == 0),
                                        stop=(ci == CCH - 1))
                                t = gpool.tile([P, 512], f32, tag=f"{nm}g")
                                d = pi * CCH + co
                                nc.vector.tensor_scalar(
                                    t[:], ps[:], rb[:, 0:1], dcol[:, d:d + 1],
                                    ALU.mult, ALU.add)
                                qk[nm, co] = t

                        # ---- VT: per loc, [64 s, 512 co] ----
                        vt = []
                        for w in range(GRP):
                            ps = pss.tile([T, 512], f32, tag="ppv")
                            for ci in range(CCH):
                                nc.tensor.matmul(ps[:], xloc(ci, w0 + w),
                                                 w_sb["v", ci][:],
                                                 start=(ci == 0), stop=False)
                            nc.tensor.matmul(ps[:], orr_sb[0:1, 0:T], dvr[:],
                                             start=False, stop=True)
                            t = gpool.tile([T, 512], f32r, tag="vtg")
                            nc.scalar.activation(t[:], ps[:], AF.Copy, bias=0.0,
                                                 scale=rb[0:T, 0:1])
                            vt.append(t)

                        # ---- scores S^T[s, (w,t)] ----
                        # one bank holds 8 independent accumulation chains, so
                        # zero it explicitly (PSUM start=True zeroes the whole
                        # 2KB bank, clobbering sibling chains) and accumulate
                        # with start=False onto the memset zeros
                        ps_s = psm.tile([T, 512], f32, tag="pss")
                        nc.vector.memset(ps_s[:], 0.0)
                        for w in range(GRP):
                            for ci in range(CCH):
                                kl = qk["k", ci][:, w * T:(w + 1) * T]
                                ql = qk["q", ci][:, w * T:(w + 1) * T]
                                nc.tensor.matmul(ps_s[:, w * T:(w + 1) * T],
                                                 kl, ql, start=False,
                                                 stop=(ci == CCH - 1),
                                                 skip_group_check=True)
                        # ---- softmax (no max-subtraction) ----
                        pexp = spool.tile([T, 512], f32r, tag="pexp")
                        nc.scalar.activation(pexp[:], ps_s[:], AF.Exp)
                        pm = spool.tile([T, 512], f32r, tag="pmask")
                        nc.vector.tensor_mul(pm[:], pexp[:].bitcast(f32),
                                             mask_sb[:])
                        ps_sum = psm.tile([1, 512], f32, tag="psum_s")
                        nc.tensor.matmul(ps_sum[:], ocr_sb[0:T, :], pm[:],
                                         start=True, stop=True)
                        rs = spool.tile([1, 512], f32r, tag="rs")
                        with nc.allow_low_precision(
                                reason="float32r is full fp32 storage"):
                            nc.vector.reciprocal(rs[:], ps_sum[:])
                        ps_rb = psm.tile([T, 512], f32, tag="psmall")
                        nc.tensor.matmul(ps_rb[:], orr_sb[0:1, 0:T], rs[:],
                                         start=True, stop=True)
                        pn = spool.tile([T, 512], f32r, tag="pn")
                        nc.vector.tensor_mul(pn[:], pm[:].bitcast(f32), ps_rb[:])

                        # ---- AV: O[c,(w,t)] ----
                        og = []
                        for ch in range(CCH):
                            ps_o = pp.tile([P, 512], f32, tag="pp")
                            nc.vector.memset(ps_o[:], 0.0)
                            for w in range(GRP):
                                lhsT = vt[w][:, ch * P:(ch + 1) * P]
                                nc.tensor.matmul(ps_o[:, w * T:(w + 1) * T],
                                                 lhsT, pn[:, w * T:(w + 1) * T],
                                                 start=False, stop=True,
                                                 skip_group_check=True)
                            t = gpool.tile([P, 512], f32r, tag="og")
                            nc.scalar.copy(t[:], ps_o[:])
                            og.append(t)

                        # ---- P-projection + bias (rank-1) + residual ----
                        for co in range(CCH):
                            ps = pp.tile([P, 512], f32, tag="pp")
                            for ci in range(CCH):
                                nc.tensor.matmul(
                                    ps[:], w_sb["p", ci][:, co * P:(co + 1) * P],
                                    og[ci][:], start=(ci == 0), stop=False)
                            nc.tensor.matmul(
                                ps[:], bprow_sb[:, co * P:(co + 1) * P],
                                orr_sb[:, 0:512], start=False, stop=True)
                            ps3 = ps[:].rearrange("p (w t) -> p w t", w=GRP)
                            xsl = xgrp(co, w0)
                            nc.vector.tensor_add(xsl, ps3, xsl.bitcast(f32))

                    for ci in range(CCH):
                        nc.sync.dma_start(outp[blk, ci * P:(ci + 1) * P, :],
                                          xb[ci][:].bitcast(f32))
    nc.compile()
    return nc


def host_prep(gamma, beta, wq, bq, wk, bk, wv, bv, wp, bp):
    """Fold gamma/beta into weights; build all constant tensors."""
    s = 1.0 / np.sqrt(np.float32(C))
    g = gamma.astype(np.float64)

    def fold(w, bias, scale):
        a = (w.astype(np.float64) * g[None, :]) * scale      # (co, ci)
        u = (w.astype(np.float64) @ g) * scale               # (co,)
        c0 = (bias.astype(np.float64) + w.astype(np.float64) @
              beta.astype(np.float64)) * scale
        return (np.ascontiguousarray(a.T.astype(np.float32)),
                u.astype(np.float32), c0.astype(np.float32))

    aqt, uq, cq = fold(wq, bq, s)
    akt, uk, ck = fold(wk, bk, 1.0)
    avt, uv, cv = fold(wv, bv, 1.0)
    apt = np.ascontiguousarray(wp.T.astype(np.float32))

    ucol = np.empty((P, 2 * CCH), np.float32)
    ccol = np.empty((P, 2 * CCH), np.float32)
    for pi, (u, c0) in enumerate(((uq, cq), (uk, ck))):
        for ch in range(CCH):
            ucol[:, pi * CCH + ch] = u[ch * P:(ch + 1) * P]
            ccol[:, pi * CCH + ch] = c0[ch * P:(ch + 1) * P]

    maskt = np.tile(np.triu(np.ones((T, T), np.float32)), (1, GRP))
    consts = {
        "wqt": aqt, "wkt": akt, "wvt": avt, "wpt": apt,
        "ucol": ucol, "ccol": ccol,
        "uvrow": uv[None, :].copy(), "cvrow": cv[None, :].copy(),
        "bprow": bp.astype(np.float32)[None, :].copy(),
        "maskt": np.ascontiguousarray(maskt),
        "ones_col_f": np.ones((P, 1), np.float32),
        "ones_col_r": np.ones((P, 1), np.float32),
        "ones_row_r": np.ones((1, C), np.float32),
    }
    return consts


_NC_CACHE = {}


def kernel(x, gamma, beta, wq, bq, wk, bk, wv, bv, wp, bp):
    x = np.asarray(x, np.float32)
    args = [np.asarray(a, np.float32) for a in
            (gamma, beta, wq, bq, wk, bk, wv, bv, wp, bp)]
    consts = host_prep(*args)

    if "nc" not in _NC_CACHE:
        _NC_CACHE["nc"] = build_nc()
    nc = _NC_CACHE["nc"]

    in_maps = []
    for core in range(NCORES):
        b, hg = core // 4, core % 4
        shard = x[b, :, :, hg * HSH:(hg + 1) * HSH, :]        # (C,T,HSH,W)
        shard = np.ascontiguousarray(
            shard.transpose(2, 0, 1, 3)).reshape(HSH, C, T * W)
        in_maps.append({"xs": shard, **consts})

    global _last_in_maps
    _last_in_maps = in_maps
    res = run_bass_kernel_spmd(nc, in_maps, list(range(NCORES)))

    out = np.empty((B, C, T, H, W), np.float32)
    for core in range(NCORES):
        b, hg = core // 4, core % 4
        o = res.results[core]["out"].reshape(HSH, C, T, W)
        out[b, :, :, hg * HSH:(hg + 1) * HSH, :] = o.transpose(1, 2, 0, 3)
    return out



# revision 4
# speedup vs baseline: 1.9275x; 1.9275x over previous
"""CausalTemporalAttnBlock Trainium2 kernel (bf16 datapath, pair-batched attn).

Problem: out = x + Wp @ attn(norm(x)) + bp, where norm is GroupNorm(1 group)
over (c,t,h,w) per batch, attention is causal over t, independent per (b,h,w).
Shapes: x (2, 512, 64, 32, 32) fp32; four (512,512) weights + biases.

Strategy (8 NeuronCores, zero communication except a 4-float AllReduce for
the GroupNorm stats):
  - core i handles batch i//4, h-rows [8*(i%4), 8*(i%4)+8), all w: 256 (h,w)
    locations per core.
  - Host re-lays the shard as [8 h][512 c][32 w * 64 t] (w-major) in BF16 so
    every matmul operand is a contiguous slice. Output written back fp32.
  - Whole datapath in bf16 (tolerance 2e-2; bf16 keeps matmuls at 1 cyc/row
    on the PE vs 4 for fp32r at moving-dim < 256, and 4x cheaper LDWEIGHTS).
  - GroupNorm stats: per-tile Square on ACT with accum_out (per-channel
    sumsq), per-tile DVE reduce for sums; 4-float AllReduce across the 4
    cores of each batch; mean/rstd math on device.
  - Normalization folded into projections, softmax-invariance exploited:
    scores ~ k_hat . (r^2 q_hat + r dq), so the K path needs NO affine at
    all (plain copy evict) and Q's r^2 is pre-multiplied into Wq once per
    kernel; V's r is pre-multiplied into Wv, V bias added as a rank-1
    matmul into PSUM.
  - Attention computed on PAIRS of locations: scores S2 = [K2]^T [Q2] gives
    a [128, 128] block per pair (diag 64x64 blocks valid); the causal mask
    multiply also zeroes the off-diagonal cross-location blocks, which makes
    both the column-sum denominators and A^T V contractions over the full
    128 partitions correct with half the matmul instructions.
  - Softmax denominators: per-pair ones-matmul column sums into rows 0..3 of
    one PSUM bank, one multi-lane reciprocal, broadcast back via 4 rank-1
    matmuls, single renormalize multiply.
"""

import numpy as np
import ml_dtypes

import concourse.bass as bass
import concourse.tile as tile
from concourse import bacc, mybir
from concourse.bass_utils import run_bass_kernel_spmd

P = 128
B, C, T, H, W = 2, 512, 64, 32, 32
NCORES = 8
HSH = H // 4          # 8 h-rows per core
CCH = C // P          # 4 c chunks
GRP = 8               # w locations per attention group
NGRP = W // GRP       # 4 groups per h-row
NPAIR = GRP // 2      # 4 location-pairs per group
TW = T * W            # 2048 tokens per (h, c-chunk) tile
GC = GRP * T          # 512 token columns per group
EPS = 1e-6

f32 = mybir.dt.float32
f32r = mybir.dt.float32r
bf16 = mybir.dt.bfloat16
AX = mybir.AxisListType.X
ALU = mybir.AluOpType
AF = mybir.ActivationFunctionType
BF = ml_dtypes.bfloat16


def build_nc(num_cores=NCORES, nblk=HSH, norm_n=None, replica_groups=None,
             reps=1, use_collective=True):
    if norm_n is None:
        norm_n = C * T * H * W
    if replica_groups is None:
        replica_groups = [[0, 1, 2, 3], [4, 5, 6, 7]]
    nc = bacc.Bacc("TRN2", target_bir_lowering=False, debug=False,
                   num_devices=num_cores)

    xs = nc.declare_dram_parameter("xs", [nblk, C, TW], bf16, isOutput=False)
    wts = {}
    for nm in ("q", "k", "v", "p"):
        wts[nm] = nc.declare_dram_parameter(f"w{nm}t", [C, C], bf16,
                                            isOutput=False)
    ucolq = nc.declare_dram_parameter("ucolq", [P, CCH], f32, isOutput=False)
    ccolq = nc.declare_dram_parameter("ccolq", [P, CCH], f32, isOutput=False)
    uvrow = nc.declare_dram_parameter("uvrow", [1, C], f32, isOutput=False)
    cvrow = nc.declare_dram_parameter("cvrow", [1, C], f32, isOutput=False)
    bprow = nc.declare_dram_parameter("bprow", [1, C], bf16, isOutput=False)
    maskp = nc.declare_dram_parameter("maskt", [P, NPAIR * P], bf16,
                                      isOutput=False)
    ones_col_f = nc.declare_dram_parameter("ones_col_f", [P, 1], f32,
                                           isOutput=False)
    ones_col_b = nc.declare_dram_parameter("ones_col_b", [P, 1], bf16,
                                           isOutput=False)
    ones_row_b = nc.declare_dram_parameter("ones_row_b", [1, C], bf16,
                                           isOutput=False)
    ones_row_r = nc.declare_dram_parameter("ones_row_r", [1, C], f32r,
                                           isOutput=False)
    outp = nc.declare_dram_parameter("out", [nblk, C, TW], f32, isOutput=True)
    cc_in = nc.dram_tensor("cc_in", [1, 2], f32)
    cc_out = nc.dram_tensor("cc_out", [1, 2], f32)

    with tile.TileContext(nc) as tc:
        with (
            tc.tile_pool(name="const", bufs=1) as const,
            tc.tile_pool(name="wscl", bufs=1) as wscl,
            tc.tile_pool(name="scal", bufs=1) as sc,
            tc.tile_pool(name="statp", bufs=3) as statp,
            tc.tile_pool(name="sqp", bufs=2) as sqp,
            tc.tile_pool(name="xpool", bufs=2) as xpool,
            tc.tile_pool(name="gpool", bufs=2) as gpool,
            tc.tile_pool(name="spool", bufs=2) as spool,
            tc.tile_pool(name="opool", bufs=3) as opool,
            tc.tile_pool(name="pp", bufs=3, space="PSUM") as pp,
            tc.tile_pool(name="pv", bufs=2, space="PSUM") as pv,
            tc.tile_pool(name="ps1", bufs=1, space="PSUM") as ps1,
            tc.tile_pool(name="psd", bufs=1, space="PSUM") as psd,
        ):
            # ---------- constants ----------
            w_sb = {}
            for nm in ("q", "k", "v", "p"):
                for ci in range(CCH):
                    t = const.tile([P, C], bf16, tag=f"w{nm}{ci}")
                    nc.sync.dma_start(t[:], wts[nm][ci * P:(ci + 1) * P, :])
                    w_sb[nm, ci] = t
            ucq_sb = const.tile([P, CCH], f32, tag="ucq")
            nc.sync.dma_start(ucq_sb[:], ucolq[:])
            ccq_sb = const.tile([P, CCH], f32, tag="ccq")
            nc.sync.dma_start(ccq_sb[:], ccolq[:])
            uvrow_sb = const.tile([1, C], f32, tag="uvrow")
            nc.sync.dma_start(uvrow_sb[:], uvrow[:])
            cvrow_sb = const.tile([1, C], f32, tag="cvrow")
            nc.sync.dma_start(cvrow_sb[:], cvrow[:])
            bprow_sb = const.tile([1, C], bf16, tag="bprow")
            nc.sync.dma_start(bprow_sb[:], bprow[:])
            mask_sb = const.tile([P, NPAIR * P], bf16, tag="maskt")
            nc.sync.dma_start(mask_sb[:], maskp[:])
            ocf_sb = const.tile([P, 1], f32, tag="ocf")
            nc.sync.dma_start(ocf_sb[:], ones_col_f[:])
            ocb_sb = const.tile([P, 1], bf16, tag="ocb")
            nc.sync.dma_start(ocb_sb[:], ones_col_b[:])
            orb_sb = const.tile([1, C], bf16, tag="orb")
            nc.sync.dma_start(orb_sb[:], ones_row_b[:])
            orr_sb = const.tile([1, C], f32r, tag="orr")
            nc.sync.dma_start(orr_sb[:], ones_row_r[:])

            for _rep in range(reps):
                # ---------- stats: sum and sumsq of the bf16 shard ----------
                ntile = nblk * CCH
                ssum = sc.tile([P, ntile], f32, tag="ssum")
                ssq = sc.tile([P, ntile], f32, tag="ssq")
                for blk in range(nblk):
                    for ci in range(CCH):
                        i = blk * CCH + ci
                        xt = statp.tile([P, TW], bf16, tag="xstat")
                        nc.sync.dma_start(
                            xt[:], xs[blk, ci * P:(ci + 1) * P, :])
                        sq = sqp.tile([P, TW], bf16, tag="sqt")
                        nc.scalar.activation(sq[:], xt[:], AF.Square,
                                             accum_out=ssq[:, i:i + 1])
                        nc.vector.reduce_sum(out=ssum[:, i:i + 1], in_=xt[:],
                                             axis=AX)
                st2 = sc.tile([P, 2], f32, tag="st2")
                nc.vector.reduce_sum(out=st2[:, 0:1], in_=ssum[:], axis=AX)
                nc.vector.reduce_sum(out=st2[:, 1:2], in_=ssq[:], axis=AX)
                ps_small = pp.tile([P, 512], f32, tag="pp")
                nc.tensor.matmul(ps_small[0:1, 0:2], ocf_sb[:], st2[:],
                                 start=True, stop=True)
                st_sb = sc.tile([1, 2], f32, tag="st_sb")
                nc.vector.tensor_copy(st_sb[:], ps_small[0:1, 0:2])
                nc.gpsimd.dma_start(cc_in[:], st_sb[:])
                if use_collective:
                    nc.gpsimd.collective_compute(
                        "AllReduce", ALU.add, replica_groups=replica_groups,
                        ins=[cc_in[:]], outs=[cc_out[:]])
                else:
                    nc.gpsimd.dma_start(cc_out[:], cc_in[:])
                stg = sc.tile([1, 2], f32, tag="stg")
                nc.gpsimd.dma_start(stg[:], cc_out[:])

                mean = sc.tile([1, 1], f32, tag="mean")
                nc.scalar.activation(mean[:], stg[:, 0:1], AF.Copy,
                                     bias=0.0, scale=1.0 / norm_n)
                ex2 = sc.tile([1, 1], f32, tag="ex2")
                nc.scalar.activation(ex2[:], stg[:, 1:2], AF.Copy,
                                     bias=0.0, scale=1.0 / norm_n)
                msq = sc.tile([1, 1], f32, tag="msq")
                nc.scalar.activation(msq[:], mean[:], AF.Square)
                varp = sc.tile([1, 1], f32, tag="varp")
                nc.vector.tensor_scalar(varp[:], ex2[:], msq[:], EPS,
                                        ALU.subtract, ALU.add)
                sqv = sc.tile([1, 1], f32, tag="sqv")      # = 1/rstd
                nc.scalar.activation(sqv[:], varp[:], AF.Sqrt)
                rst = sc.tile([1, 1], f32, tag="rst")      # = rstd
                nc.vector.reciprocal(rst[:], sqv[:])
                rmu = sc.tile([1, 1], f32, tag="rmu")      # = rstd*mean
                nc.vector.tensor_scalar(rmu[:], mean[:], rst[:], None, ALU.mult)
                vals = sc.tile([1, 2], f32r, tag="vals")
                nc.vector.tensor_copy(vals[:, 0:1], rst[:])
                nc.vector.tensor_copy(vals[:, 1:2], rmu[:])
                # broadcast (rstd, rstd*mean) across 128 partitions
                nc.tensor.matmul(ps_small[:, 0:2], orr_sb[0:1, 0:P], vals[:],
                                 start=True, stop=True)
                rb = sc.tile([P, 2], f32, tag="rb")
                nc.vector.tensor_copy(rb[:], ps_small[:, 0:2])
                rb2 = sc.tile([P, 1], f32, tag="rb2")       # r^2
                nc.vector.tensor_tensor(rb2[:], rb[:, 0:1], rb[:, 0:1],
                                        ALU.mult)
                rmr = sc.tile([P, 1], f32, tag="rmr")       # r^2 * mu
                nc.vector.tensor_tensor(rmr[:], rb[:, 0:1], rb[:, 1:2],
                                        ALU.mult)
                # q eviction bias: bqc = r*cq - (r^2 mu)*uq   (per co chunk)
                bqc = sc.tile([P, CCH], f32, tag="bqc")
                nc.vector.tensor_scalar(bqc[:], ucq_sb[:], rmr[:], None,
                                        ALU.mult)
                tqc = sc.tile([P, CCH], f32, tag="tqc")
                nc.vector.tensor_scalar(tqc[:], ccq_sb[:], rb[:, 0:1], None,
                                        ALU.mult)
                nc.vector.tensor_tensor(bqc[:], tqc[:], bqc[:], ALU.subtract)
                # V rank-1 bias row: dv = cv - rmu*uv  (bf16)
                tv0 = sc.tile([1, C], f32, tag="tv0")
                nc.vector.tensor_scalar(tv0[:], uvrow_sb[:], rmu[:], None,
                                        ALU.mult)
                nc.vector.tensor_tensor(tv0[:], cvrow_sb[:], tv0[:],
                                        ALU.subtract)
                dvr = sc.tile([1, C], bf16, tag="dvr")
                nc.vector.tensor_copy(dvr[:], tv0[:])
                # prescale weights: Wq *= r^2, Wv *= r  (uniform across rows)
                wq_s, wv_s = [], []
                for ci in range(CCH):
                    tq = wscl.tile([P, C], bf16, tag=f"wqs{ci}")
                    nc.vector.tensor_scalar(tq[:], w_sb["q", ci][:], rb2[:],
                                            None, ALU.mult)
                    wq_s.append(tq)
                    tv = wscl.tile([P, C], bf16, tag=f"wvs{ci}")
                    nc.vector.tensor_scalar(tv[:], w_sb["v", ci][:],
                                            rb[:, 0:1], None, ALU.mult)
                    wv_s.append(tv)

                # ---------- main blocks ----------
                for blk in range(nblk):
                    xb = []
                    for ci in range(CCH):
                        t = xpool.tile([P, TW], bf16, tag=f"xb{ci}")
                        nc.sync.dma_start(t[:], xs[blk, ci * P:(ci + 1) * P, :])
                        xb.append(t)

                    for g in range(NGRP):
                        c0 = g * GC
                        # ---- Q, K projections: psum[co, (w,t)] over ci ----
                        qk = {}
                        for nm, wsrc in (("q", wq_s), ("k", None)):
                            for co in range(CCH):
                                ps = pp.tile([P, 512], f32, tag="pp")
                                for ci in range(CCH):
                                    lhsT = (wsrc[ci] if wsrc is not None
                                            else w_sb["k", ci])
                                    nc.tensor.matmul(
                                        ps[:], lhsT[:, co * P:(co + 1) * P],
                                        xb[ci][:, c0:c0 + GC],
                                        start=(ci == 0), stop=(ci == CCH - 1))
                                t = gpool.tile([P, 512], bf16, tag=f"{nm}g{co}")
                                if nm == "q":
                                    nc.vector.tensor_scalar(
                                        t[:], ps[:], bqc[:, co:co + 1], None,
                                        ALU.add)
                                else:
                                    nc.scalar.copy(t[:], ps[:])
                                qk[nm, co] = t

                        # ---- VT per pair: [128=(tA,tB), 512 c] ----
                        vt = []
                        for j in range(NPAIR):
                            ps = pv.tile([P, 512], f32, tag="ppv")
                            for ci in range(CCH):
                                nc.tensor.matmul(
                                    ps[:], xb[ci][:, c0 + j * P:c0 + (j + 1) * P],
                                    wv_s[ci][:], start=(ci == 0), stop=False)
                            nc.tensor.matmul(ps[:], orb_sb[0:1, 0:P], dvr[:],
                                             start=False, stop=True)
                            t = gpool.tile([P, 512], bf16, tag=f"vtg{j}")
                            nc.scalar.copy(t[:], ps[:])
                            vt.append(t)

                        # ---- scores S2[s2, (pair, t2)] ----
                        ps_s = ps1.tile([P, 512], f32, tag="pss")
                        nc.vector.memset(ps_s[:], 0.0)
                        for j in range(NPAIR):
                            for co in range(CCH):
                                kl = qk["k", co][:, j * P:(j + 1) * P]
                                ql = qk["q", co][:, j * P:(j + 1) * P]
                                nc.tensor.matmul(ps_s[:, j * P:(j + 1) * P],
                                                 kl, ql, start=False,
                                                 stop=(co == CCH - 1),
                                                 skip_group_check=True)
                        # ---- softmax (no max-subtraction) ----
                        pexp = spool.tile([P, 512], bf16, tag="pexp")
                        nc.scalar.activation(pexp[:], ps_s[:], AF.Exp)
                        pm = spool.tile([P, 512], bf16, tag="pmask")
                        nc.vector.tensor_tensor(pm[:], pexp[:], mask_sb[:],
                                                ALU.mult)
                        # column sums (mask zeroing makes full-column sums
                        # the correct per-location denominators)
                        ps_d = psd.tile([1, 512], f32, tag="psd")
                        nc.tensor.matmul(ps_d[:], ocb_sb[:], pm[:],
                                         start=True, stop=True)
                        rsf = spool.tile([1, 512], f32, tag="rsf")
                        nc.vector.reciprocal_approx_fast(rsf[:], ps_d[:])
                        rsb = spool.tile([1, 512], bf16, tag="rsb")
                        nc.scalar.copy(rsb[:], rsf[:])
                        pb = psd.tile([P, 512], f32, tag="pb")
                        nc.tensor.matmul(pb[:], orb_sb[0:1, 0:P], rsb[:],
                                         start=True, stop=True)
                        pn = spool.tile([P, 512], bf16, tag="pn")
                        nc.vector.tensor_tensor(pn[:], pm[:], pb[:], ALU.mult)

                        # ---- AV: og[ch][c, (pair, t2)] ----
                        og = []
                        for ch in range(CCH):
                            ps_o = pp.tile([P, 512], f32, tag="pp")
                            nc.vector.memset(ps_o[:], 0.0)
                            for j in range(NPAIR):
                                lhsT = vt[j][:, ch * P:(ch + 1) * P]
                                nc.tensor.matmul(ps_o[:, j * P:(j + 1) * P],
                                                 lhsT, pn[:, j * P:(j + 1) * P],
                                                 start=False, stop=True,
                                                 skip_group_check=True)
                            t = gpool.tile([P, 512], bf16, tag=f"og{ch}")
                            nc.scalar.copy(t[:], ps_o[:])
                            og.append(t)

                        # ---- P-projection + bias (rank-1) + residual ----
                        for co in range(CCH):
                            ps = pp.tile([P, 512], f32, tag="pp")
                            for ci in range(CCH):
                                nc.tensor.matmul(
                                    ps[:], w_sb["p", ci][:, co * P:(co + 1) * P],
                                    og[ci][:], start=(ci == 0), stop=False)
                            nc.tensor.matmul(
                                ps[:], bprow_sb[:, co * P:(co + 1) * P],
                                orb_sb[:, 0:512], start=False, stop=True)
                            ot = opool.tile([P, 512], f32, tag="ot")
                            nc.vector.tensor_tensor(ot[:], ps[:],
                                                    xb[co][:, c0:c0 + GC],
                                                    ALU.add)
                            nc.sync.dma_start(
                                outp[blk, co * P:(co + 1) * P, c0:c0 + GC],
                                ot[:])
    nc.compile()
    return nc


def host_prep(gamma, beta, wq, bq, wk, bk, wv, bv, wp, bp):
    """Fold gamma/beta into weights; build all constant tensors."""
    s = 1.0 / np.sqrt(np.float32(C))
    g = gamma.astype(np.float64)

    def fold(w, bias, scale):
        a = (w.astype(np.float64) * g[None, :]) * scale      # (co, ci)
        u = (w.astype(np.float64) @ g) * scale               # (co,)
        c0 = (bias.astype(np.float64) + w.astype(np.float64) @
              beta.astype(np.float64)) * scale
        return (np.ascontiguousarray(a.T.astype(BF)),
                u.astype(np.float32), c0.astype(np.float32))

    aqt, uq, cq = fold(wq, bq, s)
    akt, uk, ck = fold(wk, bk, 1.0)
    avt, uv, cv = fold(wv, bv, 1.0)
    apt = np.ascontiguousarray(wp.T.astype(BF))

    ucolq = np.empty((P, CCH), np.float32)
    ccolq = np.empty((P, CCH), np.float32)
    for ch in range(CCH):
        ucolq[:, ch] = uq[ch * P:(ch + 1) * P]
        ccolq[:, ch] = cq[ch * P:(ch + 1) * P]

    # pair-batched causal mask: block-diag of two upper-tri (s<=t) blocks,
    # tiled across the 4 pairs of a group
    tri = np.triu(np.ones((T, T), np.float32))
    blk2 = np.zeros((P, P), np.float32)
    blk2[:T, :T] = tri
    blk2[T:, T:] = tri
    maskt = np.tile(blk2, (1, NPAIR))
    consts = {
        "wqt": aqt, "wkt": akt, "wvt": avt, "wpt": apt,
        "ucolq": ucolq, "ccolq": ccolq,
        "uvrow": uv[None, :].copy(), "cvrow": cv[None, :].copy(),
        "bprow": bp.astype(BF)[None, :].copy(),
        "maskt": np.ascontiguousarray(maskt.astype(BF)),
        "ones_col_f": np.ones((P, 1), np.float32),
        "ones_col_b": np.ones((P, 1), BF),
        "ones_row_b": np.ones((1, C), BF),
        "ones_row_r": np.ones((1, C), np.float32),
    }
    return consts


_NC_CACHE = {}


def kernel(x, gamma, beta, wq, bq, wk, bk, wv, bv, wp, bp):
    x = np.asarray(x, np.float32)
    args = [np.asarray(a, np.float32) for a in
            (gamma, beta, wq, bq, wk, bk, wv, bv, wp, bp)]
    consts = host_prep(*args)

    if "nc" not in _NC_CACHE:
        _NC_CACHE["nc"] = build_nc()
    nc = _NC_CACHE["nc"]

    in_maps = []
    for core in range(NCORES):
        b, hg = core // 4, core % 4
        shard = x[b, :, :, hg * HSH:(hg + 1) * HSH, :]        # (C,T,HSH,W)
        # -> (HSH, C, W, T) w-major, bf16
        shard = np.ascontiguousarray(
            shard.transpose(2, 0, 3, 1)).astype(BF).reshape(HSH, C, TW)
        in_maps.append({"xs": shard, **consts})

    global _last_in_maps
    _last_in_maps = in_maps
    res = run_bass_kernel_spmd(nc, in_maps, list(range(NCORES)))

    out = np.empty((B, C, T, H, W), np.float32)
    for core in range(NCORES):
        b, hg = core // 4, core % 4
        o = res.results[core]["out"].reshape(HSH, C, W, T)
        out[b, :, :, hg * HSH:(hg + 1) * HSH, :] = o.transpose(1, 3, 0, 2)
    return out


# revision 8
# speedup vs baseline: 2.3138x; 1.2004x over previous
"""CausalTemporalAttnBlock Trainium2 kernel (bf16 datapath, pair-batched attn).

Problem: out = x + Wp @ attn(norm(x)) + bp, where norm is GroupNorm(1 group)
over (c,t,h,w) per batch, attention is causal over t, independent per (b,h,w).
Shapes: x (2, 512, 64, 32, 32) fp32; four (512,512) weights + biases.

Strategy (8 NeuronCores, zero communication except a 4-float AllReduce for
the GroupNorm stats):
  - core i handles batch i//4, h-rows [8*(i%4), 8*(i%4)+8), all w: 256 (h,w)
    locations per core.
  - Host re-lays the shard as [8 h][512 c][32 w * 64 t] (w-major) in BF16 so
    every matmul operand is a contiguous slice. Output written back fp32.
  - Whole datapath in bf16 (tolerance 2e-2; bf16 keeps matmuls at 1 cyc/row
    on the PE vs 4 for fp32r at moving-dim < 256, and 4x cheaper LDWEIGHTS).
  - GroupNorm stats: per-tile Square on ACT with accum_out (per-channel
    sumsq), per-tile DVE reduce for sums; 4-float AllReduce across the 4
    cores of each batch; mean/rstd math on device.
  - All normalization folds are arranged so K projection+evict, V
    projection+evict and Q PSUM accumulation are INDEPENDENT of the stats:
    * scores ~ k_hat . (r^2 q_hat + r dq) (softmax shift invariance), so K
      needs no affine at all and Q's affine rides the eviction tensor_scalar.
    * V's r and bias dv are pushed through the linear attention+projection:
      out = x + (r Wp) @ attn_raw + (Wp dv + bp); Wp is prescaled once on
      device and (Wp dv + bp) is a once-per-kernel 16-tiny-matmul matvec
      applied via scalar_tensor_tensor at the residual add. Zero rank-1 bias
      matmuls in the main loop.
    Block 0 emits all K/V work for its 4 groups first so the PE has ~60us of
    stats-independent work to chew while the AllReduce completes.
  - Attention computed on PAIRS of locations: scores S2 = [K2]^T [Q2] gives
    a [128, 128] block per pair (diag 64x64 blocks valid); the causal mask
    multiply also zeroes the off-diagonal cross-location blocks, which makes
    both the column-sum denominators and A^T V contractions over the full
    128 partitions correct with half the matmul instructions.
  - Softmax denominators: one ones-matmul column sum, reciprocal_approx_fast
    (multi-ULP ok: uniform scale on attn weights), broadcast back via one
    rank-1 matmul, single renormalize multiply.
"""

import numpy as np
import ml_dtypes

import concourse.bass as bass
import concourse.tile as tile
from concourse import bacc, mybir
from concourse.bass_utils import run_bass_kernel_spmd

P = 128
B, C, T, H, W = 2, 512, 64, 32, 32
NCORES = 8
HSH = H // 4          # 8 h-rows per core
CCH = C // P          # 4 c chunks
GRP = 8               # w locations per attention group
NGRP = W // GRP       # 4 groups per h-row
NPAIR = GRP // 2      # 4 location-pairs per group
TW = T * W            # 2048 tokens per (h, c-chunk) tile
GC = GRP * T          # 512 token columns per group
EPS = 1e-6

f32 = mybir.dt.float32
f32r = mybir.dt.float32r
bf16 = mybir.dt.bfloat16
AX = mybir.AxisListType.X
ALU = mybir.AluOpType
AF = mybir.ActivationFunctionType
BF = ml_dtypes.bfloat16


def build_nc(num_cores=NCORES, nblk=HSH, norm_n=None, replica_groups=None,
             reps=1, use_collective=True):
    if norm_n is None:
        norm_n = C * T * H * W
    if replica_groups is None:
        replica_groups = [[0, 1, 2, 3], [4, 5, 6, 7]]
    nc = bacc.Bacc("TRN2", target_bir_lowering=False, debug=False,
                   num_devices=num_cores)

    xs = nc.declare_dram_parameter("xs", [nblk, C, TW], bf16, isOutput=False)
    wts = {}
    for nm in ("q", "k", "v", "p"):
        wts[nm] = nc.declare_dram_parameter(f"w{nm}t", [C, C], bf16,
                                            isOutput=False)
    ucolq = nc.declare_dram_parameter("ucolq", [P, CCH], f32, isOutput=False)
    ccolq = nc.declare_dram_parameter("ccolq", [P, CCH], f32, isOutput=False)
    ucolv = nc.declare_dram_parameter("ucolv", [P, CCH], f32, isOutput=False)
    ccolv = nc.declare_dram_parameter("ccolv", [P, CCH], f32, isOutput=False)
    bpcol = nc.declare_dram_parameter("bpcol", [P, CCH], f32, isOutput=False)
    maskp = nc.declare_dram_parameter("maskt", [P, NPAIR * P], bf16,
                                      isOutput=False)
    ones_col_f = nc.declare_dram_parameter("ones_col_f", [P, 1], f32,
                                           isOutput=False)
    ones_col_b = nc.declare_dram_parameter("ones_col_b", [P, 1], bf16,
                                           isOutput=False)
    ones_row_b = nc.declare_dram_parameter("ones_row_b", [1, C], bf16,
                                           isOutput=False)
    ones_row_r = nc.declare_dram_parameter("ones_row_r", [1, C], f32r,
                                           isOutput=False)
    outp = nc.declare_dram_parameter("out", [nblk, C, TW], f32, isOutput=True)
    cc_in = nc.dram_tensor("cc_in", [1, 2], f32)
    cc_out = nc.dram_tensor("cc_out", [1, 2], f32)

    with tile.TileContext(nc) as tc:
        with (
            tc.tile_pool(name="const", bufs=1) as const,
            tc.tile_pool(name="wscl", bufs=1) as wscl,
            tc.tile_pool(name="scal", bufs=1) as sc,
            tc.tile_pool(name="statp", bufs=3) as statp,
            tc.tile_pool(name="sqp", bufs=2) as sqp,
            tc.tile_pool(name="xpool", bufs=2) as xpool,
            tc.tile_pool(name="kpool", bufs=4) as kpool,
            tc.tile_pool(name="vpool", bufs=4) as vpool,
            tc.tile_pool(name="qpool", bufs=2) as qpool,
            tc.tile_pool(name="ogpool", bufs=2) as ogpool,
            tc.tile_pool(name="spool", bufs=2) as spool,
            tc.tile_pool(name="opool", bufs=3) as opool,
            tc.tile_pool(name="pp", bufs=3, space="PSUM") as pp,
            tc.tile_pool(name="pv", bufs=2, space="PSUM") as pv,
            tc.tile_pool(name="ps1", bufs=2, space="PSUM") as ps1,
            tc.tile_pool(name="psd", bufs=1, space="PSUM") as psd,
        ):
            # ---------- constants ----------
            w_sb = {}
            for nm in ("q", "k", "v", "p"):
                for ci in range(CCH):
                    t = const.tile([P, C], bf16, tag=f"w{nm}{ci}")
                    nc.sync.dma_start(t[:], wts[nm][ci * P:(ci + 1) * P, :])
                    w_sb[nm, ci] = t
            ucq_sb = const.tile([P, CCH], f32, tag="ucq")
            nc.sync.dma_start(ucq_sb[:], ucolq[:])
            ccq_sb = const.tile([P, CCH], f32, tag="ccq")
            nc.sync.dma_start(ccq_sb[:], ccolq[:])
            ucv_sb = const.tile([P, CCH], f32, tag="ucv")
            nc.sync.dma_start(ucv_sb[:], ucolv[:])
            ccv_sb = const.tile([P, CCH], f32, tag="ccv")
            nc.sync.dma_start(ccv_sb[:], ccolv[:])
            bpc_sb = const.tile([P, CCH], f32, tag="bpc0")
            nc.sync.dma_start(bpc_sb[:], bpcol[:])
            mask_sb = const.tile([P, NPAIR * P], bf16, tag="maskt")
            nc.sync.dma_start(mask_sb[:], maskp[:])
            ocf_sb = const.tile([P, 1], f32, tag="ocf")
            nc.sync.dma_start(ocf_sb[:], ones_col_f[:])
            ocb_sb = const.tile([P, 1], bf16, tag="ocb")
            nc.sync.dma_start(ocb_sb[:], ones_col_b[:])
            orb_sb = const.tile([1, C], bf16, tag="orb")
            nc.sync.dma_start(orb_sb[:], ones_row_b[:])
            orr_sb = const.tile([1, C], f32r, tag="orr")
            nc.sync.dma_start(orr_sb[:], ones_row_r[:])

            for _rep in range(reps):
                # ---------- stats: sum and sumsq of the bf16 shard ----------
                ntile = nblk * CCH
                ssum = sc.tile([P, ntile], f32, tag="ssum")
                ssq = sc.tile([P, ntile], f32, tag="ssq")
                for blk in range(nblk):
                    for ci in range(CCH):
                        i = blk * CCH + ci
                        xt = statp.tile([P, TW], bf16, tag="xstat")
                        nc.sync.dma_start(
                            xt[:], xs[blk, ci * P:(ci + 1) * P, :])
                        sq = sqp.tile([P, TW], bf16, tag="sqt")
                        nc.scalar.activation(sq[:], xt[:], AF.Square,
                                             accum_out=ssq[:, i:i + 1])
                        nc.vector.reduce_sum(out=ssum[:, i:i + 1], in_=xt[:],
                                             axis=AX)
                st2 = sc.tile([P, 2], f32, tag="st2")
                nc.vector.reduce_sum(out=st2[:, 0:1], in_=ssum[:], axis=AX)
                nc.vector.reduce_sum(out=st2[:, 1:2], in_=ssq[:], axis=AX)
                # stats/prologue PSUM tiles live in the ps1 ring: its later
                # users (scores/pb) are post-collective anyway, so they never
                # block the stats-independent K/V chains rotating through pp.
                ps_small = ps1.tile([P, 512], f32, tag="pss")
                nc.tensor.matmul(ps_small[0:1, 0:2], ocf_sb[:], st2[:],
                                 start=True, stop=True)
                st_sb = sc.tile([1, 2], f32, tag="st_sb")
                nc.vector.tensor_copy(st_sb[:], ps_small[0:1, 0:2])
                nc.gpsimd.dma_start(cc_in[:], st_sb[:])
                if use_collective:
                    nc.gpsimd.collective_compute(
                        "AllReduce", ALU.add, replica_groups=replica_groups,
                        ins=[cc_in[:]], outs=[cc_out[:]])
                else:
                    nc.gpsimd.dma_start(cc_out[:], cc_in[:])
                stg = sc.tile([1, 2], f32, tag="stg")
                nc.gpsimd.dma_start(stg[:], cc_out[:])

                mean = sc.tile([1, 1], f32, tag="mean")
                nc.scalar.activation(mean[:], stg[:, 0:1], AF.Copy,
                                     bias=0.0, scale=1.0 / norm_n)
                ex2 = sc.tile([1, 1], f32, tag="ex2")
                nc.scalar.activation(ex2[:], stg[:, 1:2], AF.Copy,
                                     bias=0.0, scale=1.0 / norm_n)
                msq = sc.tile([1, 1], f32, tag="msq")
                nc.scalar.activation(msq[:], mean[:], AF.Square)
                varp = sc.tile([1, 1], f32, tag="varp")
                nc.vector.tensor_scalar(varp[:], ex2[:], msq[:], EPS,
                                        ALU.subtract, ALU.add)
                sqv = sc.tile([1, 1], f32, tag="sqv")      # = 1/rstd
                nc.scalar.activation(sqv[:], varp[:], AF.Sqrt)
                rst = sc.tile([1, 1], f32, tag="rst")      # = rstd
                nc.vector.reciprocal(rst[:], sqv[:])
                rmu = sc.tile([1, 1], f32, tag="rmu")      # = rstd*mean
                nc.vector.tensor_scalar(rmu[:], mean[:], rst[:], None, ALU.mult)
                vals = sc.tile([1, 2], f32r, tag="vals")
                nc.vector.tensor_copy(vals[:, 0:1], rst[:])
                nc.vector.tensor_copy(vals[:, 1:2], rmu[:])
                # broadcast (rstd, rstd*mean) across 128 partitions
                ps_rb = ps1.tile([P, 512], f32, tag="pss")
                nc.tensor.matmul(ps_rb[:, 0:2], orr_sb[0:1, 0:P], vals[:],
                                 start=True, stop=True)
                rb = sc.tile([P, 2], f32, tag="rb")
                nc.vector.tensor_copy(rb[:], ps_rb[:, 0:2])
                rb2 = sc.tile([P, 1], f32, tag="rb2")       # r^2
                nc.vector.tensor_tensor(rb2[:], rb[:, 0:1], rb[:, 0:1],
                                        ALU.mult)
                rmr = sc.tile([P, 1], f32, tag="rmr")       # r^2 * mu
                nc.vector.tensor_tensor(rmr[:], rb[:, 0:1], rb[:, 1:2],
                                        ALU.mult)
                # q eviction bias: bqc = r*cq - (r^2 mu)*uq   (per co chunk)
                bqc = sc.tile([P, CCH], f32, tag="bqc")
                nc.vector.tensor_scalar(bqc[:], ucq_sb[:], rmr[:], None,
                                        ALU.mult)
                tqc = sc.tile([P, CCH], f32, tag="tqc")
                nc.vector.tensor_scalar(tqc[:], ccq_sb[:], rb[:, 0:1], None,
                                        ALU.mult)
                nc.vector.tensor_tensor(bqc[:], tqc[:], bqc[:], ALU.subtract)
                # V bias column: dv = cv - rmu*uv  (per ci chunk, bf16)
                dvc = sc.tile([P, CCH], f32, tag="dvc")
                nc.vector.tensor_scalar(dvc[:], ucv_sb[:], rb[:, 1:2], None,
                                        ALU.mult)
                nc.vector.tensor_tensor(dvc[:], ccv_sb[:], dvc[:],
                                        ALU.subtract)
                dvb = sc.tile([P, CCH], bf16, tag="dvb")
                nc.vector.tensor_copy(dvb[:], dvc[:])
                # prescale Wp by r (the V-path r folded through attention)
                wp_s = []
                for ci in range(CCH):
                    tw = wscl.tile([P, C], bf16, tag=f"wps{ci}")
                    nc.vector.tensor_scalar(tw[:], w_sb["p", ci][:],
                                            rb[:, 0:1], None, ALU.mult)
                    wp_s.append(tw)
                # residual bias column: bpc = bp + Wp @ dv   (per co chunk)
                pw = ps1.tile([P, 512], f32, tag="pss")
                nc.vector.memset(pw[:, 0:CCH], 0.0)
                for co in range(CCH):
                    for ci in range(CCH):
                        nc.tensor.matmul(
                            pw[:, co:co + 1],
                            w_sb["p", ci][:, co * P:(co + 1) * P],
                            dvb[:, ci:ci + 1], start=False,
                            stop=(ci == CCH - 1), skip_group_check=True)
                bpc = sc.tile([P, CCH], f32, tag="bpct")
                nc.vector.tensor_tensor(bpc[:], pw[:, 0:CCH], bpc_sb[:],
                                        ALU.add)

                # ---------- main blocks ----------
                for blk in range(nblk):
                    xb = []
                    for ci in range(CCH):
                        t = xpool.tile([P, TW], bf16, tag=f"xb{ci}")
                        nc.sync.dma_start(t[:], xs[blk, ci * P:(ci + 1) * P, :])
                        xb.append(t)

                    kt = {}
                    vt = {}

                    def emit_k(g):
                        c0 = g * GC
                        for co in range(CCH):
                            ps = pp.tile([P, 512], f32, tag="pp")
                            for ci in range(CCH):
                                nc.tensor.matmul(
                                    ps[:], w_sb["k", ci][:, co * P:(co + 1) * P],
                                    xb[ci][:, c0:c0 + GC],
                                    start=(ci == 0), stop=(ci == CCH - 1))
                            t = kpool.tile([P, 512], bf16, tag=f"kg{co}")
                            nc.scalar.copy(t[:], ps[:])
                            kt[g, co] = t

                    def emit_vt(g):
                        c0 = g * GC
                        for j in range(NPAIR):
                            ps = pv.tile([P, 512], f32, tag="ppv")
                            for ci in range(CCH):
                                nc.tensor.matmul(
                                    ps[:],
                                    xb[ci][:, c0 + j * P:c0 + (j + 1) * P],
                                    w_sb["v", ci][:],
                                    start=(ci == 0), stop=(ci == CCH - 1))
                            t = vpool.tile([P, 512], bf16, tag=f"vtg{j}")
                            nc.scalar.copy(t[:], ps[:])
                            vt[g, j] = t

                    def emit_q_attn(g):
                        c0 = g * GC
                        qt = []
                        for co in range(CCH):
                            ps = pp.tile([P, 512], f32, tag="pp")
                            for ci in range(CCH):
                                nc.tensor.matmul(
                                    ps[:], w_sb["q", ci][:, co * P:(co + 1) * P],
                                    xb[ci][:, c0:c0 + GC],
                                    start=(ci == 0), stop=(ci == CCH - 1))
                            t = qpool.tile([P, 512], bf16, tag=f"qg{co}")
                            nc.vector.tensor_scalar(
                                t[:], ps[:], rb2[:], bqc[:, co:co + 1],
                                ALU.mult, ALU.add)
                            qt.append(t)

                        # ---- scores S2[s2, (pair, t2)] ----
                        ps_s = ps1.tile([P, 512], f32, tag="pss")
                        for j in range(NPAIR):
                            for co in range(CCH):
                                kl = kt[g, co][:, j * P:(j + 1) * P]
                                ql = qt[co][:, j * P:(j + 1) * P]
                                nc.tensor.matmul(ps_s[:, j * P:(j + 1) * P],
                                                 kl, ql,
                                                 start=(j == 0 and co == 0),
                                                 stop=(co == CCH - 1),
                                                 skip_group_check=True)
                        # ---- softmax (no max-subtraction) ----
                        pexp = spool.tile([P, 512], bf16, tag="pexp")
                        nc.scalar.activation(pexp[:], ps_s[:], AF.Exp)
                        pm = spool.tile([P, 512], bf16, tag="pmask")
                        nc.vector.tensor_tensor(pm[:], pexp[:], mask_sb[:],
                                                ALU.mult)
                        ps_d = psd.tile([1, 512], f32, tag="psd")
                        nc.tensor.matmul(ps_d[:], ocb_sb[:], pm[:],
                                         start=True, stop=True)
                        rsf = spool.tile([1, 512], f32, tag="rsf")
                        nc.vector.reciprocal_approx_fast(rsf[:], ps_d[:])
                        rsb = spool.tile([1, 512], bf16, tag="rsb")
                        nc.scalar.copy(rsb[:], rsf[:])
                        pb = ps1.tile([P, 512], f32, tag="pss")
                        nc.tensor.matmul(pb[:], orb_sb[0:1, 0:P], rsb[:],
                                         start=True, stop=True)
                        pn = spool.tile([P, 512], bf16, tag="pn")
                        nc.vector.tensor_tensor(pn[:], pm[:], pb[:], ALU.mult)

                        # ---- AV: og[ch][c, (pair, t2)] ----
                        og = []
                        for ch in range(CCH):
                            ps_o = pp.tile([P, 512], f32, tag="pp")
                            for j in range(NPAIR):
                                lhsT = vt[g, j][:, ch * P:(ch + 1) * P]
                                nc.tensor.matmul(ps_o[:, j * P:(j + 1) * P],
                                                 lhsT, pn[:, j * P:(j + 1) * P],
                                                 start=(j == 0), stop=True,
                                                 skip_group_check=True)
                            t = ogpool.tile([P, 512], bf16, tag=f"og{ch}")
                            nc.scalar.copy(t[:], ps_o[:])
                            og.append(t)

                        # ---- P-projection + residual (+ folded biases) ----
                        for co in range(CCH):
                            ps = pp.tile([P, 512], f32, tag="pp")
                            for ci in range(CCH):
                                nc.tensor.matmul(
                                    ps[:], wp_s[ci][:, co * P:(co + 1) * P],
                                    og[ci][:], start=(ci == 0),
                                    stop=(ci == CCH - 1))
                            ot = opool.tile([P, 512], f32, tag="ot")
                            nc.vector.scalar_tensor_tensor(
                                ot[:], ps[:], bpc[:, co:co + 1],
                                xb[co][:, c0:c0 + GC], ALU.add, ALU.add)
                            nc.sync.dma_start(
                                outp[blk, co * P:(co + 1) * P, c0:c0 + GC],
                                ot[:])

                    if blk == 0:
                        # All stats-independent K/V work first: the PE chews
                        # this while the stats AllReduce completes.
                        for g in range(NGRP):
                            emit_k(g)
                            emit_vt(g)
                        for g in range(NGRP):
                            emit_q_attn(g)
                    else:
                        for g in range(NGRP):
                            emit_k(g)
                            emit_vt(g)
                            emit_q_attn(g)
    nc.compile()
    return nc


def host_prep(gamma, beta, wq, bq, wk, bk, wv, bv, wp, bp):
    """Fold gamma/beta into weights; build all constant tensors."""
    s = 1.0 / np.sqrt(np.float32(C))
    g = gamma.astype(np.float64)

    def fold(w, bias, scale):
        a = (w.astype(np.float64) * g[None, :]) * scale      # (co, ci)
        u = (w.astype(np.float64) @ g) * scale               # (co,)
        c0 = (bias.astype(np.float64) + w.astype(np.float64) @
              beta.astype(np.float64)) * scale
        return (np.ascontiguousarray(a.T.astype(BF)),
                u.astype(np.float32), c0.astype(np.float32))

    aqt, uq, cq = fold(wq, bq, s)
    akt, uk, ck = fold(wk, bk, 1.0)
    avt, uv, cv = fold(wv, bv, 1.0)
    apt = np.ascontiguousarray(wp.T.astype(BF))

    def col(v):
        return np.ascontiguousarray(
            v.astype(np.float32).reshape(CCH, P).T)

    # pair-batched causal mask: block-diag of two upper-tri (s<=t) blocks,
    # tiled across the 4 pairs of a group
    tri = np.triu(np.ones((T, T), np.float32))
    blk2 = np.zeros((P, P), np.float32)
    blk2[:T, :T] = tri
    blk2[T:, T:] = tri
    maskt = np.tile(blk2, (1, NPAIR))
    consts = {
        "wqt": aqt, "wkt": akt, "wvt": avt, "wpt": apt,
        "ucolq": col(uq), "ccolq": col(cq),
        "ucolv": col(uv), "ccolv": col(cv),
        "bpcol": col(bp),
        "maskt": np.ascontiguousarray(maskt.astype(BF)),
        "ones_col_f": np.ones((P, 1), np.float32),
        "ones_col_b": np.ones((P, 1), BF),
        "ones_row_b": np.ones((1, C), BF),
        "ones_row_r": np.ones((1, C), np.float32),
    }
    return consts


_NC_CACHE = {}


def kernel(x, gamma, beta, wq, bq, wk, bk, wv, bv, wp, bp):
    x = np.asarray(x, np.float32)
    args = [np.asarray(a, np.float32) for a in
            (gamma, beta, wq, bq, wk, bk, wv, bv, wp, bp)]
    consts = host_prep(*args)

    if "nc" not in _NC_CACHE:
        _NC_CACHE["nc"] = build_nc()
    nc = _NC_CACHE["nc"]

    in_maps = []
    for core in range(NCORES):
        b, hg = core // 4, core % 4
        shard = x[b, :, :, hg * HSH:(hg + 1) * HSH, :]        # (C,T,HSH,W)
        # -> (HSH, C, W, T) w-major, bf16
        shard = np.ascontiguousarray(
            shard.transpose(2, 0, 3, 1)).astype(BF).reshape(HSH, C, TW)
        in_maps.append({"xs": shard, **consts})

    global _last_in_maps
    _last_in_maps = in_maps
    res = run_bass_kernel_spmd(nc, in_maps, list(range(NCORES)))

    out = np.empty((B, C, T, H, W), np.float32)
    for core in range(NCORES):
        b, hg = core // 4, core % 4
        o = res.results[core]["out"].reshape(HSH, C, W, T)
        out[b, :, :, hg * HSH:(hg + 1) * HSH, :] = o.transpose(1, 3, 0, 2)
    return out
